# revision 25
# baseline (speedup 1.0000x reference)
"""Trainium2 Bass kernel for rank-1 attention + linear (nn_Attention).

Reference computation (S=256, B=128, D=4096):
    scores   = einsum('sbd,bd->bs', inp, hidden[0])      # dot each enc state with hidden
    attn     = softmax(scores, axis=1)                   # over S
    weighted = einsum('bs,sbd->bd', attn, inp)
    concat   = [weighted, hidden[0]]   # [B, 2D]
    out      = concat @ W.T + b        # [1, B, D]

Distribution over 8 NeuronCores:
  - attention part: data-parallel over B (16 batches per core)
  - linear part: W sharded over output dim (512 rows per core); NORMALIZED,
    TRANSPOSED weighted vectors exchanged with two on-chip AllGathers
    (batches 0-7 / 8-15) so only the second exchange's latency is exposed.

All heavy operands are f16 (host-cast). Timeline model from the baseline
trace (281us): ~16us preamble, then a DMA-saturated main phase (~320 GB/s),
then the exchange+linear tail. v2 changes vs the 287us baseline:
  - e-vectors are normalized by the softmax denominator BEFORE the weighted
    matmuls (ACT scale-copy with per-partition recip; denominator comes free
    from the Exp instruction's accum_out). Kills the ones-column den matmul,
    den evac, and the entire post-exchange normalize pipeline.
  - sender-side PE transposes repack each group's weighted sums to d-major
    [128, 256] f16 before the exchange, so the post-AllGather tail is ONLY
    32 matmuls + bias + store per group (no normalize/transpose/cast).
  - scores: one DVE tensor_tensor product per s-tile (fp16 2x mode), with
    the free-dim sum split ACT-accum [0:XACT] / DVE tensor_reduce [XACT:]
    balanced to measured rates (ACT 1 elem/cy @1.2G incl 224cy init + 280ns
    accum-read; DVE TR 1 elem/cy @0.96G; DVE TT 2 elem/cy).
  - bias folded into the linear's PSUM accumulation via a ones-row matmul
    (drops gpsimd partition_broadcast and the gpsimd library load).
  - weighted accumulation uses 4 partition-base quadrants (0/32/64/96) x
    1024 d-cols = 2 PSUM banks (single group, reused A/B), freeing banks.
  - nat DMAs lead the sync queue; wt/hT streaming starts at b==1 so the
    first batch's products aren't queued behind weight traffic.

Per-core dataflow:
  hb      : hidden pair-rows broadcast to all 128 partitions straight
            from HBM via a 0-stride-partition DMA (channel-staggered copies)
  scores  : DVE tensor_tensor product; ACT accum + DVE reduce split
  softmax : max via PE transposes of the per-pair partial-sum columns,
            exp on ACT (accum_out = denominator), recip + scale-copy to
            get normalized e-vectors, PE transpose back to column form
  weighted: PE matmuls with column-masked f16 e-vectors into the 4-quadrant
            wacc; evac'd per 8-batch group, PE-transposed to d-major, and
            AllGathered as [128, 256] f16
  linear  : hidden half from host-pretransposed hidT during the loop; bias
            via ones-row matmul; weighted half from gathered d-major tiles
            (strided [128,8slots,8batch] lhsT), psum->sbuf copy, store.
            Out rows are in exchange order (g,k,j) -> b = k*16+g*8+j; the
            host un-permutes.
"""

import sys

if "/opt/trn_rl_repo" not in sys.path:
    sys.path.insert(0, "/opt/trn_rl_repo")

import numpy as np


# ----------------------------------------------------------------------------
# Program builder
# ----------------------------------------------------------------------------

def build_program(S=256, B=128, D=4096, n_cores=8):
    import concourse.bacc as bacc
    import concourse.mybir as mybir
    import concourse.tile as tile

    f32 = mybir.dt.float32
    f16 = mybir.dt.float16
    P = 128
    Bc = B // n_cores                 # batches per core (16)
    ST = S // P                       # s-tiles per batch (2)
    DOUT = D // n_cores               # output-dim shard per core (512)
    NKF = 2 * D // P                  # 128-wide k-chunks of the linear (64)
    ND = D // P                       # 128-wide d-chunks (32)
    G = Bc // 2                       # batch pairs (8)
    XACT = 3072                       # ACT's share of each 4096-elem row sum

    nc = bacc.Bacc(None, target_bir_lowering=False)

    inp = nc.dram_tensor("inp", [Bc, ST, P, D], f16, kind="ExternalInput")
    # hidden pair-rows, 2 copies with a channel-phase-staggered stride
    HBW = 10240
    hid = nc.dram_tensor("hid", [2, G, HBW], f16, kind="ExternalInput")
    hT = nc.dram_tensor("hT", [P, ND, P], f16, kind="ExternalInput")
    wt = nc.dram_tensor("wt", [P, NKF, DOUT], f16, kind="ExternalInput")
    biasr = nc.dram_tensor("biasr", [1, DOUT], f16, kind="ExternalInput")
    ident = nc.dram_tensor("ident", [P, P], f32, kind="ExternalInput")
    # 8x8 f16 identities at partition bases 0/32/64/96 (col-transposes +
    # sender-side [8,128] transposes)
    idf3 = nc.dram_tensor("idf3", [P, 8], f16, kind="ExternalInput")
    onesbc = nc.dram_tensor("onesbc", [P, P], f16, kind="ExternalInput")
    out = nc.dram_tensor("out", [B, DOUT], f32, kind="ExternalOutput")

    WCC = 2 * P                       # exchange payload cols (d-major f16)
    cc_in = [nc.dram_tensor(f"cc_in{g}", [P, WCC], f16) for g in range(2)]
    cc_out = [
        nc.dram_tensor(f"cc_out{g}", [n_cores * P, WCC], f16, addr_space="Shared")
        for g in range(2)
    ]

    with tile.TileContext(nc) as tc:
        import contextlib

        with contextlib.ExitStack() as ctx:
            persist = ctx.enter_context(tc.tile_pool(name="persist", bufs=1))

            # ---- small prefetches on the ACT (scalar) HWDGE queue ----
            ident_sb = persist.tile([P, P], f32)
            nc.scalar.dma_start(out=ident_sb, in_=ident[:, :])
            idf3_sb = persist.tile([P, 8], f16)
            nc.scalar.dma_start(out=idf3_sb, in_=idf3[:, :])
            onesbc_sb = persist.tile([P, P], f16)
            nc.scalar.dma_start(out=onesbc_sb, in_=onesbc[:, :])
            biasr_sb = persist.tile([1, DOUT], f16)
            nc.scalar.dma_start(out=biasr_sb, in_=biasr[:, :])
            # hT and wt stream during the loop (b==1/3/8/10) so the head's
            # sync-queue nat loads aren't bandwidth-starved
            hT_sb = persist.tile([P, ND, P], f16)
            wt_sb = persist.tile([P, NKF, DOUT], f16)

            # masked e-vectors: [s, t, col] per 8-batch group; col j of slice
            # (t, j) holds batch (grp*8+j)'s normalized e-values, else zero
            diag = persist.tile([P, ST, 8, 8], f16)
            nc.vector.memset(diag[:, :, :, :].bitcast(f32), 0.0)

            # normalized weighted sums, f16, evac dest: 3 strips of
            # [8, w] at partition bases 0/32/64 (w = 1536/1536/1024)
            wsn = persist.tile([P, 2, 1536], f16)
            # d-major exchange payload per group: [128, 2*128] f16
            wT_loc = persist.tile([P, 2, WCC], f16)

            # PSUM: wacc 2 banks (4 quadrants x 1024 d-cols, groups reuse),
            # lin 2 banks: out accumulator cols 0:512, score-transpose
            # scratch 512:768, e-columns 768:770, d-major transpose scratch
            # f16 cols 776:904
            linp = ctx.enter_context(tc.tile_pool(name="lin", bufs=1, space="PSUM"))
            lin_ps = linp.tile([P, 1024], f32)
            out_ps = lin_ps[:, 0:DOUT]
            scT3 = lin_ps[0:2, DOUT : DOUT + 2 * P]
            ebs = [
                lin_ps[:, DOUT + 2 * P + t : DOUT + 2 * P + t + 1].bitcast(f16)
                for t in range(ST)
            ]
            trps = lin_ps[:, 776:904].bitcast(f16)  # [P, 256] f16

            # (base, out_col, d_lo); n=512 (psum bank limit). Matmul out
            # partition bases may only be 0/32/64, so the 4096 d-cols are
            # spread 1536/1536/1024 over those bases.
            MM_CHUNKS = [
                (0, 0, 0),
                (0, 512, 512),
                (0, 1024, 1024),
                (32, 0, 1536),
                (32, 512, 2048),
                (32, 1024, 2560),
                (64, 0, 3072),
                (64, 512, 3584),
            ]
            STRIPS = [(0, 1536, 0), (32, 1536, 1536), (64, 1024, 3072)]

            loop_stack = ctx.enter_context(contextlib.ExitStack())
            natp = loop_stack.enter_context(tc.tile_pool(name="nat", bufs=4))
            hbp = loop_stack.enter_context(tc.tile_pool(name="hb", bufs=2))
            prodp = loop_stack.enter_context(tc.tile_pool(name="prod", bufs=3))
            smalls = loop_stack.enter_context(tc.tile_pool(name="smalls", bufs=3))

            wacc_stack = ctx.enter_context(contextlib.ExitStack())
            waccp = wacc_stack.enter_context(
                tc.tile_pool(name="wacc", bufs=1, space="PSUM")
            )
            wacc = waccp.tile([P, 1536], f32)

            def emit_hb(g):
                # broadcast hidden rows 2g,2g+1 to all partitions straight
                # from HBM: 0-stride partition dim -> 64 same-row descriptors
                # per copy
                hb = hbp.tile([P, 2 * D], f16, tag="hb")
                nc.sync.dma_start(
                    out=hb, in_=hid[:, g, 0 : 2 * D].partition_broadcast(P // 2)
                )
                return hb

            def evac_group(g):
                # wacc strips -> normalized f16 strips (already e-norm'd)
                for base, width, _ in STRIPS:
                    nc.scalar.activation(
                        out=wsn[base : base + 8, g, 0:width],
                        in_=wacc[base : base + 8, 0:width],
                        func=mybir.ActivationFunctionType.Copy,
                    )
                # repack to d-major [128, 256]: chunk c covers d 128c:128c+128
                for c in range(ND):
                    base, _, d_lo = STRIPS[min(c // 12, 2)]
                    cc = c - d_lo // P
                    nc.tensor.transpose(
                        trps[:, c * 8 : (c + 1) * 8],
                        wsn[base : base + 8, g, cc * P : (cc + 1) * P],
                        idf3_sb[base : base + 8, 0:8],
                    )
                nc.vector.tensor_copy(wT_loc[:, g, :], trps)
                nc.scalar.dma_start(out=cc_in[g][:, :], in_=wT_loc[:, g, :])

            def emit_allgather(g):
                nc.gpsimd.collective_compute(
                    "AllGather",
                    mybir.AluOpType.bypass,
                    replica_groups=[list(range(n_cores))],
                    ins=[cc_in[g][:, :]],
                    outs=[cc_out[g][:, :]],
                )

            # ---------------- attention (batch loop) ----------------
            hbs = {0: emit_hb(0)}
            nats = {}
            sc2 = None

            for b in range(Bc):
                grp, j = divmod(b, 8)
                g2 = b // 2

                nat = natp.tile([P, ST, D], f16, tag="nat")
                for t in range(ST):
                    nc.sync.dma_start(out=nat[:, t, :], in_=inp[b, t])
                nats[b] = nat

                # stream wt in 2MB chunks: hidden half early, weighted late
                if b in (1, 3, 8, 10):
                    q = {1: 2, 3: 3, 8: 0, 10: 1}[b]
                    nc.scalar.dma_start(
                        out=wt_sb[:, q * 16 : (q + 1) * 16, :],
                        in_=wt[:, q * 16 : (q + 1) * 16, :],
                    )
                if b == 1:
                    nc.scalar.dma_start(out=hT_sb, in_=hT[:, :, :])

                hb = hbs[g2][:, (b % 2) * D : (b % 2 + 1) * D]
                if b % 2 == 0 and g2 + 1 < G:
                    hbs[g2 + 1] = emit_hb(g2 + 1)

                if b % 2 == 0:
                    sc2 = smalls.tile([P, 8], f32, tag="sc")
                for t in range(ST):
                    # fp16 tensor_tensor runs the DVE 2x mode; the free-dim
                    # sum is split ACT-accum [0:XACT] / DVE reduce [XACT:]
                    prod = prodp.tile([P, D], f16, tag="prod")
                    nc.vector.tensor_tensor(
                        out=prod, in0=nat[:, t, :], in1=hb, op=mybir.AluOpType.mult
                    )
                    c0 = 4 * (b % 2) + 2 * t
                    nc.scalar.activation(
                        out=prod[:, 0:XACT],
                        in_=prod[:, 0:XACT],
                        func=mybir.ActivationFunctionType.Copy,
                        accum_out=sc2[:, c0 : c0 + 1],
                    )
                    nc.vector.tensor_reduce(
                        out=sc2[:, c0 + 1 : c0 + 2],
                        in_=prod[:, XACT:D],
                        axis=mybir.AxisListType.X,
                        op=mybir.AluOpType.add,
                    )

                if b % 2 == 1:
                    # pair softmax via PE transposes: combine partials,
                    # transpose scores to rows=batch, per-batch max on DVE,
                    # exp on ACT (accum_out = denominator), normalize the
                    # e-rows, transpose back for the masked matmul columns
                    sc3 = smalls.tile([P, 4], f32, tag="sc3")
                    nc.vector.tensor_tensor(
                        out=sc3,
                        in0=sc2[:, 0:8:2],
                        in1=sc2[:, 1:8:2],
                        op=mybir.AluOpType.add,
                    )
                    nc.tensor.transpose(scT3[:, 0:P], sc3[:, 0:4:2], ident_sb)
                    nc.tensor.transpose(scT3[:, P : 2 * P], sc3[:, 1:4:2], ident_sb)
                    negm2 = smalls.tile([2, 1], f32, tag="negm")
                    nc.vector.tensor_reduce(
                        out=negm2, in_=scT3[:, 0 : 2 * P], axis=mybir.AxisListType.X,
                        op=mybir.AluOpType.max, negate=True,
                    )
                    eT2 = smalls.tile([2, 2 * P], f16, tag="eT")
                    den2 = smalls.tile([2, 1], f32, tag="den")
                    nc.scalar.activation(
                        out=eT2,
                        in_=scT3[:, 0 : 2 * P],
                        func=mybir.ActivationFunctionType.Exp,
                        bias=negm2,
                        scale=1.0,
                        accum_out=den2,
                    )
                    recip2 = smalls.tile([2, 1], f32, tag="recip")
                    nc.vector.reciprocal(recip2, den2)
                    eT2n = smalls.tile([2, 2 * P], f16, tag="eTn")
                    nc.scalar.activation(
                        out=eT2n,
                        in_=eT2,
                        func=mybir.ActivationFunctionType.Copy,
                        scale=recip2,
                    )
                    for t in range(ST):
                        nc.tensor.transpose(
                            ebs[t], eT2n[:, t * P : (t + 1) * P], idf3_sb[0:2, 0:2]
                        )

                    # weighted-sum matmuls for both batches of the pair
                    for bb in (b - 1, b):
                        gg, jj = divmod(bb, 8)
                        r = bb - (b - 1)
                        natb = nats.pop(bb)
                        for t in range(ST):
                            nc.vector.tensor_copy(
                                diag[:, t, jj, jj : jj + 1], ebs[t][:, r : r + 1]
                            )
                        for t in range(ST):
                            lhsT = diag[:, t, jj, :]
                            st = jj == 0 and t == 0
                            sp = jj == 7 and t == ST - 1
                            for base, col, d_lo in MM_CHUNKS:
                                nc.tensor.matmul(
                                    wacc[base : base + 8, col : col + 512],
                                    lhsT,
                                    natb[:, t, d_lo : d_lo + 512],
                                    start=st,
                                    stop=sp,
                                )

                    # hidden half of the linear, spread over pairs 1..7
                    p2 = b // 2
                    if p2 >= 1:
                        for i in range(4 * (p2 - 1), 4 * (p2 - 1) + 4):
                            nc.tensor.matmul(
                                out_ps,
                                hT_sb[:, i, :],
                                wt_sb[:, ND + i, :],
                                start=(i == 0),
                                stop=False,
                                skip_group_check=True,
                            )

                    # evac + exchange per 8-batch group, right when its
                    # accumulation stops (group 1 reuses wacc's banks)
                    if b == 7:
                        evac_group(0)
                        emit_allgather(0)
                    if b == Bc - 1:
                        evac_group(1)
                        emit_allgather(1)
                        wacc_stack.close()
                        loop_stack.close()

            # ---------------- linear tail (weighted half) ----------------
            with contextlib.ExitStack() as lin_ctx:
                tailp = lin_ctx.enter_context(tc.tile_pool(name="tail", bufs=2))

                # last 4 hidden-half k-chunks (pairs only cover 0..27)
                for i in range(28, 32):
                    nc.tensor.matmul(
                        out_ps,
                        hT_sb[:, i, :],
                        wt_sb[:, ND + i, :],
                        start=False,
                        stop=False,
                        skip_group_check=True,
                    )
                # bias for all 128 rows via a ones-row matmul
                nc.tensor.matmul(
                    out_ps,
                    onesbc_sb[0:1, :],
                    biasr_sb[0:1, :],
                    start=False,
                    stop=False,
                    skip_group_check=True,
                )

                out_sb = tailp.tile([P, DOUT], f32)

                # per exchange group (rows g*64:(g+1)*64 of the permuted B)
                for g in range(2):
                    r0 = g * 64
                    wTall = tailp.tile([P, n_cores, WCC], f16, tag="wTall")
                    for s in range(n_cores):
                        nc.sync.dma_start(
                            out=wTall[:, s, :],
                            in_=cc_out[g][s * P : (s + 1) * P, :],
                        )
                    # repack slot-major -> chunk-major so each chunk's lhsT
                    # [(s,j) cols] is contiguous (walrus rejects strided
                    # weight APs)
                    wT2 = tailp.tile([P, ND, n_cores, 8], f16, tag="wT2")
                    for s in range(n_cores):
                        nc.vector.tensor_copy(
                            wT2[:, :, s, :],
                            wTall[:, s, :].rearrange("p (c j) -> p c j", c=ND),
                        )
                    wT2f = wT2.rearrange("p c s j -> p c (s j)")
                    for c in range(ND):
                        nc.tensor.matmul(
                            out_ps[r0 : r0 + 64, :],
                            wT2f[:, c, :],
                            wt_sb[:, c, :],
                            start=False,
                            stop=(c == ND - 1),
                            skip_group_check=True,
                        )
                    nc.vector.tensor_copy(
                        out_sb[r0 : r0 + 64, :], out_ps[r0 : r0 + 64, :]
                    )
                    nc.sync.dma_start(
                        out=out[r0 : r0 + 64, :], in_=out_sb[r0 : r0 + 64, :]
                    )

    nc.finalize()
    return nc


_CACHE = {}


def _get_program(S, B, D, n_cores):
    key = (S, B, D, n_cores)
    if key not in _CACHE:
        _CACHE[key] = build_program(S, B, D, n_cores)
    return _CACHE[key]


def _hid_copies(hid_k, hbw=10240):
    """Pair-rows in 2 channel-phase-staggered copies: [2, 8, hbw]."""
    g, d2 = hid_k.shape[0] // 2, hid_k.shape[1] * 2
    out = np.zeros((2, g, hbw), dtype=hid_k.dtype)
    out[:, :, 0:d2] = hid_k.reshape(g, d2)[None]
    return out


def make_in_maps(inp, hidden, W, b, n_cores=8):
    """Shard host inputs into per-core input maps (f16 for heavy operands)."""
    f16 = np.float16
    S, B, D = inp.shape
    Bc = B // n_cores
    DOUT = W.shape[0] // n_cores
    P = 128

    # batch permutation of the exchange order: i=(g,k,j) -> b = k*16+g*8+j
    perm = [k * Bc + g * 8 + j for g in range(2) for k in range(n_cores) for j in range(8)]
    hTg = np.ascontiguousarray(hidden[0].T.astype(f16))          # [D, B]
    hT_pi = hTg[:, perm]                                          # [D, B]
    hT_pack = np.ascontiguousarray(
        hT_pi.reshape(D // P, P, B).transpose(1, 0, 2)
    )                                                             # [P, ND, B]

    ident = np.eye(P, dtype=np.float32)
    idf3 = np.zeros((P, 8), dtype=f16)
    for q in range(4):
        for i in range(8):
            idf3[32 * q + i, i] = 1.0
    onesbc = np.ones((P, P), dtype=f16)

    in_maps = []
    for k in range(n_cores):
        inp_k = inp[:, k * Bc : (k + 1) * Bc, :]                  # [S, Bc, D]
        inp_pack = np.ascontiguousarray(
            inp_k.transpose(1, 0, 2).reshape(Bc, 2, P, D).astype(f16)
        )
        wtk = W[k * DOUT : (k + 1) * DOUT, :].T                   # [F, DOUT]
        wt_pack = np.ascontiguousarray(
            wtk.reshape(2 * D // P, P, DOUT).transpose(1, 0, 2).astype(f16)
        )                                                         # [P, NKF, DOUT]
        in_maps.append(
            {
                "inp": inp_pack,
                "hid": _hid_copies(
                    hidden[0, k * Bc : (k + 1) * Bc, :].astype(f16)
                ),
                "hT": hT_pack,
                "wt": wt_pack,
                "biasr": np.ascontiguousarray(
                    b[k * DOUT : (k + 1) * DOUT].reshape(1, DOUT).astype(f16)
                ),
                "ident": ident,
                "idf3": idf3,
                "onesbc": onesbc,
            }
        )
    return in_maps


def kernel(inp, hidden, W, b, trace=False):
    from concourse.bass_utils import run_bass_kernel_spmd

    inp = np.asarray(inp, dtype=np.float32)
    hidden = np.asarray(hidden, dtype=np.float32)
    W = np.asarray(W, dtype=np.float32)
    b = np.asarray(b, dtype=np.float32)

    S, B, D = inp.shape
    n_cores = 8
    nc = _get_program(S, B, D, n_cores)
    in_maps = make_in_maps(inp, hidden, W, b, n_cores)
    res = run_bass_kernel_spmd(nc, in_maps, core_ids=list(range(n_cores)))
    # per-core out rows are in exchange order i=(g,k,j) <-> b=k*16+g*8+j
    outs = [
        np.asarray(res.results[k]["out"])
        .reshape(2, n_cores, 8, -1)
        .transpose(1, 0, 2, 3)
        .reshape(B, -1)
        for k in range(n_cores)
    ]
    full = np.concatenate(outs, axis=1)  # [B, D]
    if trace:
        return full[None, :, :], res
    return full[None, :, :]


# revision 26
# speedup vs baseline: 1.0302x; 1.0302x over previous
"""Trainium2 Bass kernel for rank-1 attention + linear (nn_Attention).

Reference computation (S=256, B=128, D=4096):
    scores   = einsum('sbd,bd->bs', inp, hidden[0])      # dot each enc state with hidden
    attn     = softmax(scores, axis=1)                   # over S
    weighted = einsum('bs,sbd->bd', attn, inp)
    concat   = [weighted, hidden[0]]   # [B, 2D]
    out      = concat @ W.T + b        # [1, B, D]

Distribution over 8 NeuronCores:
  - attention part: data-parallel over B (16 batches per core)
  - linear part: W sharded over output dim (512 rows per core); NORMALIZED,
    TRANSPOSED weighted vectors exchanged with two on-chip AllGathers
    (batches 0-7 / 8-15) so only the second exchange's latency is exposed.

All heavy operands are f16 (host-cast). Timeline model from the baseline
trace (281us): ~16us preamble, then a DMA-saturated main phase (~320 GB/s),
then the exchange+linear tail. v2 changes vs the 287us baseline:
  - e-vectors are normalized by the softmax denominator BEFORE the weighted
    matmuls (ACT scale-copy with per-partition recip; denominator comes free
    from the Exp instruction's accum_out). Kills the ones-column den matmul,
    den evac, and the entire post-exchange normalize pipeline.
  - sender-side PE transposes repack each group's weighted sums to d-major
    [128, 256] f16 before the exchange, so the post-AllGather tail is ONLY
    32 matmuls + bias + store per group (no normalize/transpose/cast).
  - scores: one DVE tensor_tensor product per s-tile (fp16 2x mode), with
    the free-dim sum split ACT-accum [0:XACT] / DVE tensor_reduce [XACT:]
    balanced to measured rates (ACT 1 elem/cy @1.2G incl 224cy init + 280ns
    accum-read; DVE TR 1 elem/cy @0.96G; DVE TT 2 elem/cy).
  - bias folded into the linear's PSUM accumulation via a ones-row matmul
    (drops gpsimd partition_broadcast and the gpsimd library load).
  - weighted accumulation uses 4 partition-base quadrants (0/32/64/96) x
    1024 d-cols = 2 PSUM banks (single group, reused A/B), freeing banks.
  - nat DMAs lead the sync queue; wt/hT streaming starts at b==1 so the
    first batch's products aren't queued behind weight traffic.

Per-core dataflow:
  hb      : hidden pair-rows broadcast to all 128 partitions straight
            from HBM via a 0-stride-partition DMA (channel-staggered copies)
  scores  : DVE tensor_tensor product; ACT accum + DVE reduce split
  softmax : max via PE transposes of the per-pair partial-sum columns,
            exp on ACT (accum_out = denominator), recip + scale-copy to
            get normalized e-vectors, PE transpose back to column form
  weighted: PE matmuls with column-masked f16 e-vectors into the 4-quadrant
            wacc; evac'd per 8-batch group, PE-transposed to d-major, and
            AllGathered as [128, 256] f16
  linear  : hidden half from host-pretransposed hidT during the loop; bias
            via ones-row matmul; weighted half from gathered d-major tiles
            (strided [128,8slots,8batch] lhsT), psum->sbuf copy, store.
            Out rows are in exchange order (g,k,j) -> b = k*16+g*8+j; the
            host un-permutes.
"""

import sys

if "/opt/trn_rl_repo" not in sys.path:
    sys.path.insert(0, "/opt/trn_rl_repo")

import numpy as np


# ----------------------------------------------------------------------------
# Program builder
# ----------------------------------------------------------------------------

def build_program(S=256, B=128, D=4096, n_cores=8):
    import concourse.bacc as bacc
    import concourse.mybir as mybir
    import concourse.tile as tile

    f32 = mybir.dt.float32
    f16 = mybir.dt.float16
    P = 128
    Bc = B // n_cores                 # batches per core (16)
    ST = S // P                       # s-tiles per batch (2)
    DOUT = D // n_cores               # output-dim shard per core (512)
    NKF = 2 * D // P                  # 128-wide k-chunks of the linear (64)
    ND = D // P                       # 128-wide d-chunks (32)
    G = Bc // 2                       # batch pairs (8)
    XACT = 2944                       # ACT's share of each 4096-elem row sum

    nc = bacc.Bacc(None, target_bir_lowering=False)

    inp = nc.dram_tensor("inp", [Bc, ST, P, D], f16, kind="ExternalInput")
    # hidden pair-rows, 2 copies with a channel-phase-staggered stride
    HBW = 10240
    hid = nc.dram_tensor("hid", [2, G, HBW], f16, kind="ExternalInput")
    hT = nc.dram_tensor("hT", [P, ND, P], f16, kind="ExternalInput")
    wt = nc.dram_tensor("wt", [P, NKF, DOUT], f16, kind="ExternalInput")
    biasr = nc.dram_tensor("biasr", [1, DOUT], f16, kind="ExternalInput")
    ident = nc.dram_tensor("ident", [P, P], f32, kind="ExternalInput")
    # 8x8 f16 identities at partition bases 0/32/64/96 (col-transposes +
    # sender-side [8,128] transposes)
    idf3 = nc.dram_tensor("idf3", [P, 8], f16, kind="ExternalInput")
    onesbc = nc.dram_tensor("onesbc", [P, P], f16, kind="ExternalInput")
    out = nc.dram_tensor("out", [B, DOUT], f32, kind="ExternalOutput")

    WCC = 2 * P                       # exchange payload cols (d-major f16)
    cc_in = [nc.dram_tensor(f"cc_in{g}", [P, WCC], f16) for g in range(2)]
    cc_out = [
        nc.dram_tensor(f"cc_out{g}", [n_cores * P, WCC], f16, addr_space="Shared")
        for g in range(2)
    ]

    with tile.TileContext(nc) as tc:
        import contextlib

        with contextlib.ExitStack() as ctx:
            persist = ctx.enter_context(tc.tile_pool(name="persist", bufs=1))

            # ---- small prefetches on the ACT (scalar) HWDGE queue ----
            ident_sb = persist.tile([P, P], f32)
            nc.scalar.dma_start(out=ident_sb, in_=ident[:, :])
            idf3_sb = persist.tile([P, 8], f16)
            nc.scalar.dma_start(out=idf3_sb, in_=idf3[:, :])
            onesbc_sb = persist.tile([P, P], f16)
            nc.scalar.dma_start(out=onesbc_sb, in_=onesbc[:, :])
            biasr_sb = persist.tile([1, DOUT], f16)
            nc.scalar.dma_start(out=biasr_sb, in_=biasr[:, :])
            # hT and wt stream during the loop (b==1/3/8/10) so the head's
            # sync-queue nat loads aren't bandwidth-starved
            hT_sb = persist.tile([P, ND, P], f16)
            wt_sb = persist.tile([P, NKF, DOUT], f16)

            # masked e-vectors: [s, t, col] per 8-batch group; col j of slice
            # (t, j) holds batch (grp*8+j)'s normalized e-values, else zero
            diag = persist.tile([P, ST, 8, 8], f16)
            nc.vector.memset(diag[:, :, :, :].bitcast(f32), 0.0)

            # normalized weighted sums, f16, evac dest: 3 strips of
            # [8, w] at partition bases 0/32/64 (w = 1536/1536/1024)
            wsn = persist.tile([P, 2, 1536], f16)
            # d-major exchange payload per group: [128, 2*128] f16
            wT_loc = persist.tile([P, 2, WCC], f16)

            # PSUM: wacc 2 banks (4 quadrants x 1024 d-cols, groups reuse),
            # lin 2 banks: out accumulator cols 0:512, score-transpose
            # scratch 512:768, e-columns 768:770, d-major transpose scratch
            # f16 cols 776:904
            linp = ctx.enter_context(tc.tile_pool(name="lin", bufs=1, space="PSUM"))
            lin_ps = linp.tile([P, 1024], f32)
            out_ps = lin_ps[:, 0:DOUT]
            scT3 = lin_ps[0:2, DOUT : DOUT + 2 * P]
            ebs = [
                lin_ps[:, DOUT + 2 * P + t : DOUT + 2 * P + t + 1].bitcast(f16)
                for t in range(ST)
            ]
            trps = lin_ps[:, 776:904].bitcast(f16)  # [P, 256] f16

            # (base, out_col, d_lo); n=512 (psum bank limit). Matmul out
            # partition bases may only be 0/32/64, so the 4096 d-cols are
            # spread 1536/1536/1024 over those bases.
            MM_CHUNKS = [
                (0, 0, 0),
                (0, 512, 512),
                (0, 1024, 1024),
                (32, 0, 1536),
                (32, 512, 2048),
                (32, 1024, 2560),
                (64, 0, 3072),
                (64, 512, 3584),
            ]
            STRIPS = [(0, 1536, 0), (32, 1536, 1536), (64, 1024, 3072)]

            loop_stack = ctx.enter_context(contextlib.ExitStack())
            natp = loop_stack.enter_context(tc.tile_pool(name="nat", bufs=4))
            hbp = loop_stack.enter_context(tc.tile_pool(name="hb", bufs=2))
            prodp = loop_stack.enter_context(tc.tile_pool(name="prod", bufs=3))
            smalls = loop_stack.enter_context(tc.tile_pool(name="smalls", bufs=3))

            wacc_stack = ctx.enter_context(contextlib.ExitStack())
            waccp = wacc_stack.enter_context(
                tc.tile_pool(name="wacc", bufs=1, space="PSUM")
            )
            wacc = waccp.tile([P, 1536], f32)

            def emit_hb(g):
                # broadcast hidden rows 2g,2g+1 to all partitions straight
                # from HBM: 0-stride partition dim -> 64 same-row descriptors
                # per copy
                hb = hbp.tile([P, 2 * D], f16, tag="hb")
                nc.sync.dma_start(
                    out=hb, in_=hid[:, g, 0 : 2 * D].partition_broadcast(P // 2)
                )
                return hb

            def evac_group(g):
                # wacc strips -> normalized f16 strips (already e-norm'd)
                for base, width, _ in STRIPS:
                    nc.scalar.activation(
                        out=wsn[base : base + 8, g, 0:width],
                        in_=wacc[base : base + 8, 0:width],
                        func=mybir.ActivationFunctionType.Copy,
                    )
                # repack to d-major [128, 256]: chunk c covers d 128c:128c+128
                for c in range(ND):
                    base, _, d_lo = STRIPS[min(c // 12, 2)]
                    cc = c - d_lo // P
                    nc.tensor.transpose(
                        trps[:, c * 8 : (c + 1) * 8],
                        wsn[base : base + 8, g, cc * P : (cc + 1) * P],
                        idf3_sb[base : base + 8, 0:8],
                    )
                nc.vector.tensor_copy(wT_loc[:, g, :], trps)
                nc.scalar.dma_start(out=cc_in[g][:, :], in_=wT_loc[:, g, :])

            def emit_allgather(g):
                nc.gpsimd.collective_compute(
                    "AllGather",
                    mybir.AluOpType.bypass,
                    replica_groups=[list(range(n_cores))],
                    ins=[cc_in[g][:, :]],
                    outs=[cc_out[g][:, :]],
                )

            # ---------------- attention (batch loop) ----------------
            hbs = {0: emit_hb(0)}
            nats = {}
            sc2 = None

            for b in range(Bc):
                grp, j = divmod(b, 8)
                g2 = b // 2

                nat = natp.tile([P, ST, D], f16, tag="nat")
                for t in range(ST):
                    nc.sync.dma_start(out=nat[:, t, :], in_=inp[b, t])
                nats[b] = nat

                # stream wt in 2MB chunks: hidden half early, weighted late
                if b in (1, 3, 8, 10):
                    q = {1: 2, 3: 3, 8: 0, 10: 1}[b]
                    nc.scalar.dma_start(
                        out=wt_sb[:, q * 16 : (q + 1) * 16, :],
                        in_=wt[:, q * 16 : (q + 1) * 16, :],
                    )
                if b == 1:
                    nc.scalar.dma_start(out=hT_sb, in_=hT[:, :, :])

                hb = hbs[g2][:, (b % 2) * D : (b % 2 + 1) * D]
                if b % 2 == 0 and g2 + 1 < G:
                    hbs[g2 + 1] = emit_hb(g2 + 1)

                if b % 2 == 0:
                    sc2 = smalls.tile([P, 8], f32, tag="sc")
                for t in range(ST):
                    # fp16 tensor_tensor runs the DVE 2x mode; the free-dim
                    # sum is split ACT-accum [0:XACT] / DVE reduce [XACT:]
                    prod = prodp.tile([P, D], f16, tag="prod")
                    nc.vector.tensor_tensor(
                        out=prod, in0=nat[:, t, :], in1=hb, op=mybir.AluOpType.mult
                    )
                    c0 = 4 * (b % 2) + 2 * t
                    nc.scalar.activation(
                        out=prod[:, 0:XACT],
                        in_=prod[:, 0:XACT],
                        func=mybir.ActivationFunctionType.Copy,
                        accum_out=sc2[:, c0 : c0 + 1],
                    )
                    nc.vector.tensor_reduce(
                        out=sc2[:, c0 + 1 : c0 + 2],
                        in_=prod[:, XACT:D],
                        axis=mybir.AxisListType.X,
                        op=mybir.AluOpType.add,
                    )

                if b % 2 == 1:
                    # pair softmax via PE transposes: combine partials,
                    # transpose scores to rows=batch, per-batch max on DVE,
                    # exp on ACT (accum_out = denominator), normalize the
                    # e-rows, transpose back for the masked matmul columns
                    sc3 = smalls.tile([P, 4], f32, tag="sc3")
                    nc.vector.tensor_tensor(
                        out=sc3,
                        in0=sc2[:, 0:8:2],
                        in1=sc2[:, 1:8:2],
                        op=mybir.AluOpType.add,
                    )
                    nc.tensor.transpose(scT3[:, 0:P], sc3[:, 0:4:2], ident_sb)
                    nc.tensor.transpose(scT3[:, P : 2 * P], sc3[:, 1:4:2], ident_sb)
                    negm2 = smalls.tile([2, 1], f32, tag="negm")
                    nc.vector.tensor_reduce(
                        out=negm2, in_=scT3[:, 0 : 2 * P], axis=mybir.AxisListType.X,
                        op=mybir.AluOpType.max, negate=True,
                    )
                    eT2 = smalls.tile([2, 2 * P], f16, tag="eT")
                    den2 = smalls.tile([2, 1], f32, tag="den")
                    nc.scalar.activation(
                        out=eT2,
                        in_=scT3[:, 0 : 2 * P],
                        func=mybir.ActivationFunctionType.Exp,
                        bias=negm2,
                        scale=1.0,
                        accum_out=den2,
                    )
                    recip2 = smalls.tile([2, 1], f32, tag="recip")
                    nc.vector.reciprocal(recip2, den2)
                    eT2n = smalls.tile([2, 2 * P], f16, tag="eTn")
                    nc.scalar.activation(
                        out=eT2n,
                        in_=eT2,
                        func=mybir.ActivationFunctionType.Copy,
                        scale=recip2,
                    )
                    for t in range(ST):
                        nc.tensor.transpose(
                            ebs[t], eT2n[:, t * P : (t + 1) * P], idf3_sb[0:2, 0:2]
                        )

                    # weighted-sum matmuls for both batches of the pair
                    for bb in (b - 1, b):
                        gg, jj = divmod(bb, 8)
                        r = bb - (b - 1)
                        natb = nats.pop(bb)
                        for t in range(ST):
                            nc.vector.tensor_copy(
                                diag[:, t, jj, jj : jj + 1], ebs[t][:, r : r + 1]
                            )
                        for t in range(ST):
                            lhsT = diag[:, t, jj, :]
                            st = jj == 0 and t == 0
                            sp = jj == 7 and t == ST - 1
                            for base, col, d_lo in MM_CHUNKS:
                                nc.tensor.matmul(
                                    wacc[base : base + 8, col : col + 512],
                                    lhsT,
                                    natb[:, t, d_lo : d_lo + 512],
                                    start=st,
                                    stop=sp,
                                )

                    # hidden half of the linear, spread over pairs 1..7
                    p2 = b // 2
                    if p2 >= 1:
                        for i in range(4 * (p2 - 1), 4 * (p2 - 1) + 4):
                            nc.tensor.matmul(
                                out_ps,
                                hT_sb[:, i, :],
                                wt_sb[:, ND + i, :],
                                start=(i == 0),
                                stop=False,
                                skip_group_check=True,
                            )

                    # evac + exchange per 8-batch group, right when its
                    # accumulation stops (group 1 reuses wacc's banks)
                    if b == 7:
                        evac_group(0)
                        emit_allgather(0)
                    if b == Bc - 1:
                        evac_group(1)
                        emit_allgather(1)
                        wacc_stack.close()
                        loop_stack.close()

            # ---------------- linear tail (weighted half) ----------------
            with contextlib.ExitStack() as lin_ctx:
                tailp = lin_ctx.enter_context(tc.tile_pool(name="tail", bufs=2))

                # last 4 hidden-half k-chunks (pairs only cover 0..27)
                for i in range(28, 32):
                    nc.tensor.matmul(
                        out_ps,
                        hT_sb[:, i, :],
                        wt_sb[:, ND + i, :],
                        start=False,
                        stop=False,
                        skip_group_check=True,
                    )
                # bias for all 128 rows via a ones-row matmul
                nc.tensor.matmul(
                    out_ps,
                    onesbc_sb[0:1, :],
                    biasr_sb[0:1, :],
                    start=False,
                    stop=False,
                    skip_group_check=True,
                )

                out_sb = tailp.tile([P, DOUT], f32)

                # per exchange group (rows g*64:(g+1)*64 of the permuted B)
                for g in range(2):
                    r0 = g * 64
                    wTall = tailp.tile([P, n_cores, WCC], f16, tag="wTall")
                    for s in range(n_cores):
                        nc.sync.dma_start(
                            out=wTall[:, s, :],
                            in_=cc_out[g][s * P : (s + 1) * P, :],
                        )
                    # repack slot-major -> chunk-major so each chunk's lhsT
                    # [(s,j) cols] is contiguous (walrus rejects strided
                    # weight APs)
                    wT2 = tailp.tile([P, ND, n_cores, 8], f16, tag="wT2")
                    for s in range(n_cores):
                        nc.vector.tensor_copy(
                            wT2[:, :, s, :],
                            wTall[:, s, :].rearrange("p (c j) -> p c j", c=ND),
                        )
                    wT2f = wT2.rearrange("p c s j -> p c (s j)")
                    for c in range(ND):
                        nc.tensor.matmul(
                            out_ps[r0 : r0 + 64, :],
                            wT2f[:, c, :],
                            wt_sb[:, c, :],
                            start=False,
                            stop=(c == ND - 1),
                            skip_group_check=True,
                        )
                    nc.vector.tensor_copy(
                        out_sb[r0 : r0 + 64, :], out_ps[r0 : r0 + 64, :]
                    )
                    nc.sync.dma_start(
                        out=out[r0 : r0 + 64, :], in_=out_sb[r0 : r0 + 64, :]
                    )

    nc.finalize()
    return nc


_CACHE = {}


def _get_program(S, B, D, n_cores):
    key = (S, B, D, n_cores)
    if key not in _CACHE:
        _CACHE[key] = build_program(S, B, D, n_cores)
    return _CACHE[key]


def _hid_copies(hid_k, hbw=10240):
    """Pair-rows in 2 channel-phase-staggered copies: [2, 8, hbw]."""
    g, d2 = hid_k.shape[0] // 2, hid_k.shape[1] * 2
    out = np.zeros((2, g, hbw), dtype=hid_k.dtype)
    out[:, :, 0:d2] = hid_k.reshape(g, d2)[None]
    return out


def make_in_maps(inp, hidden, W, b, n_cores=8):
    """Shard host inputs into per-core input maps (f16 for heavy operands)."""
    f16 = np.float16
    S, B, D = inp.shape
    Bc = B // n_cores
    DOUT = W.shape[0] // n_cores
    P = 128

    # batch permutation of the exchange order: i=(g,k,j) -> b = k*16+g*8+j
    perm = [k * Bc + g * 8 + j for g in range(2) for k in range(n_cores) for j in range(8)]
    hTg = np.ascontiguousarray(hidden[0].T.astype(f16))          # [D, B]
    hT_pi = hTg[:, perm]                                          # [D, B]
    hT_pack = np.ascontiguousarray(
        hT_pi.reshape(D // P, P, B).transpose(1, 0, 2)
    )                                                             # [P, ND, B]

    ident = np.eye(P, dtype=np.float32)
    idf3 = np.zeros((P, 8), dtype=f16)
    for q in range(4):
        for i in range(8):
            idf3[32 * q + i, i] = 1.0
    onesbc = np.ones((P, P), dtype=f16)

    in_maps = []
    for k in range(n_cores):
        inp_k = inp[:, k * Bc : (k + 1) * Bc, :]                  # [S, Bc, D]
        inp_pack = np.ascontiguousarray(
            inp_k.transpose(1, 0, 2).reshape(Bc, 2, P, D).astype(f16)
        )
        wtk = W[k * DOUT : (k + 1) * DOUT, :].T                   # [F, DOUT]
        wt_pack = np.ascontiguousarray(
            wtk.reshape(2 * D // P, P, DOUT).transpose(1, 0, 2).astype(f16)
        )                                                         # [P, NKF, DOUT]
        in_maps.append(
            {
                "inp": inp_pack,
                "hid": _hid_copies(
                    hidden[0, k * Bc : (k + 1) * Bc, :].astype(f16)
                ),
                "hT": hT_pack,
                "wt": wt_pack,
                "biasr": np.ascontiguousarray(
                    b[k * DOUT : (k + 1) * DOUT].reshape(1, DOUT).astype(f16)
                ),
                "ident": ident,
                "idf3": idf3,
                "onesbc": onesbc,
            }
        )
    return in_maps


def kernel(inp, hidden, W, b, trace=False):
    from concourse.bass_utils import run_bass_kernel_spmd

    inp = np.asarray(inp, dtype=np.float32)
    hidden = np.asarray(hidden, dtype=np.float32)
    W = np.asarray(W, dtype=np.float32)
    b = np.asarray(b, dtype=np.float32)

    S, B, D = inp.shape
    n_cores = 8
    nc = _get_program(S, B, D, n_cores)
    in_maps = make_in_maps(inp, hidden, W, b, n_cores)
    res = run_bass_kernel_spmd(nc, in_maps, core_ids=list(range(n_cores)))
    # per-core out rows are in exchange order i=(g,k,j) <-> b=k*16+g*8+j
    outs = [
        np.asarray(res.results[k]["out"])
        .reshape(2, n_cores, 8, -1)
        .transpose(1, 0, 2, 3)
        .reshape(B, -1)
        for k in range(n_cores)
    ]
    full = np.concatenate(outs, axis=1)  # [B, D]
    if trace:
        return full[None, :, :], res
    return full[None, :, :]
